# revision 1
# baseline (speedup 1.0000x reference)
"""DeLanNet inverse-dynamics kernel for 8 Trainium2 NeuronCores.

tau = qDD^T (L L^T) where L is lower-triangular built from two small MLPs
of q. Data-parallel over batch; feature-major layout on device (samples in
the free dimension), bf16 matmuls with fp32 PSUM accumulation.

Decomposition of the scatter + L L^T + contraction (verified vs reference):
  e   = [h_ld; h_lo]  (28 rows)          -- MLP layer-2 output
  qg  = M1 @ qDD      (M1[m,i] = [i(m)==i], gathers row index of L-entry m)
  w   = e * qg
  vg  = S @ w         (S[m,m'] = [k(m)==k(m')])
  u   = e * vg
  tau = M1^T @ u
"""

import numpy as np
import ml_dtypes

DOF = 7
TRIL = 21
HID = 256
B = 524288
NCORES = 8
BS = B // NCORES          # 65536 samples per core
TS = 512                  # samples per tile (one PSUM bank)
SCS = 8192                # samples per DMA super-chunk
NT = SCS // TS            # 16 tiles per super-chunk
NSC = BS // SCS           # 8 super-chunks

BF16 = ml_dtypes.bfloat16

_cache = {}


def _structure():
    rows, cols = np.tril_indices(DOF, -1)
    i_m = np.concatenate([np.arange(DOF), rows])      # 28
    k_m = np.concatenate([np.arange(DOF), cols])
    M1 = np.zeros((28, DOF), np.float32)
    M1[np.arange(28), i_m] = 1.0
    S = (k_m[:, None] == k_m[None, :]).astype(np.float32)
    return M1, S


def _build_weights(W1d, b1d, W2d, b2d, W1o, b1o, W2o, b2o):
    M1, S = _structure()
    # L1: lhsT rows 0-6 = W1cat, row 7 = bias (ones row in rhs)
    W1cat = np.concatenate([W1d, W1o], axis=1)            # [7, 512]
    w1 = np.concatenate([W1cat, np.concatenate([b1d, b1o])[None, :]], 0)  # [8,512]
    # L2: block-diagonal [512, 32]: rows 0-255 d-net, 256-511 o-net; M padded to 32
    W2blk = np.zeros((2 * HID, 32), np.float32)
    W2blk[:HID, 0:DOF] = W2d
    W2blk[HID:, DOF:28] = W2o
    # device layout: [128, 128], cols 32c:32c+32 = K-chunk c
    w2r = W2blk.reshape(4, 128, 32).transpose(1, 0, 2).reshape(128, 128)
    b2r = np.zeros((128, 1), np.float32)
    for j in range(4):
        b2r[32 * j:32 * j + 28, 0] = np.concatenate([b2d, b2o])
    mqg = np.zeros((DOF, 32), np.float32)                  # lhsT: qg = M1 @ qDD
    mqg[:, :28] = M1.T
    SP = np.zeros((32, 32), np.float32)
    SP[:28, :28] = S
    mvg = np.zeros((128, 128), np.float32)                 # lhsT: vg4 block-diag
    for j in range(4):
        mvg[32 * j:32 * j + 32, 32 * j:32 * j + 32] = SP
    mtau = np.zeros((128, 28), np.float32)                 # lhsT: tau4
    for j in range(4):
        mtau[32 * j:32 * j + 28, 7 * j:7 * j + 7] = M1
    return dict(
        w1=np.ascontiguousarray(w1, np.float32),
        w2r=np.ascontiguousarray(w2r.astype(BF16)),
        b2r=b2r,
        mqg=np.ascontiguousarray(mqg.astype(BF16)),
        mvg=np.ascontiguousarray(mvg.astype(BF16)),
        mtau=np.ascontiguousarray(mtau.astype(BF16)),
    )


def _build_nc():
    import concourse.bacc as bacc
    import concourse.mybir as mybir
    from concourse.bass import ts
    from concourse.tile import TileContext

    f32 = mybir.dt.float32
    bf16 = mybir.dt.bfloat16
    AF = mybir.ActivationFunctionType

    nc = bacc.Bacc("TRN2")
    xq = nc.dram_tensor("xq", (8, BS), f32, kind="ExternalInput")
    xqd = nc.dram_tensor("xqd", (DOF, BS), f32, kind="ExternalInput")
    w1 = nc.dram_tensor("w1", (8, 512), f32, kind="ExternalInput")
    w2r = nc.dram_tensor("w2r", (128, 128), bf16, kind="ExternalInput")
    b2r = nc.dram_tensor("b2r", (128, 1), f32, kind="ExternalInput")
    mqg = nc.dram_tensor("mqg", (DOF, 32), bf16, kind="ExternalInput")
    mvg = nc.dram_tensor("mvg", (128, 128), bf16, kind="ExternalInput")
    mtau = nc.dram_tensor("mtau", (128, 28), bf16, kind="ExternalInput")
    outT = nc.dram_tensor("outT", (28, BS // 4), f32, kind="ExternalOutput")

    with TileContext(nc) as tc:
        with (
            tc.tile_pool(name="const", bufs=1) as cpool,
            tc.tile_pool(name="xin", bufs=2) as xpool,
            tc.tile_pool(name="work", bufs=2) as wpool,
            tc.tile_pool(name="oring", bufs=2) as opool,
            tc.tile_pool(name="ph", bufs=1, space="PSUM") as ph,
            tc.tile_pool(name="pmisc", bufs=1, space="PSUM") as pm,
        ):
            # --- constants to SBUF ---
            w1_sb = cpool.tile([128, 512], bf16, tag="w1")
            for j in range(4):
                nc.gpsimd.dma_start(w1_sb[32 * j:32 * j + 8, :], w1[:, :])
            w2_sb = cpool.tile([128, 128], bf16, tag="w2")
            nc.sync.dma_start(w2_sb[:, :], w2r[:, :])
            b2_sb = cpool.tile([128, 1], f32, tag="b2")
            nc.sync.dma_start(b2_sb[:, :], b2r[:, :])
            mqg_sb = cpool.tile([DOF, 32], bf16, tag="mqg")
            nc.sync.dma_start(mqg_sb[:, :], mqg[:, :])
            mvg_sb = cpool.tile([128, 128], bf16, tag="mvg")
            nc.sync.dma_start(mvg_sb[:, :], mvg[:, :])
            mtau_sb = cpool.tile([128, 28], bf16, tag="mtau")
            nc.sync.dma_start(mtau_sb[:, :], mtau[:, :])

            for sc in range(NSC):
                x0 = sc * SCS
                # input super-chunk: cast f32->bf16 during DMA (SWDGE)
                xq_sb = xpool.tile([128, SCS], bf16, tag="xq")
                nc.gpsimd.dma_start(xq_sb[0:8, :], xq[:, x0:x0 + SCS])
                for j in range(1, 4):
                    nc.sync.dma_start(xq_sb[32 * j:32 * j + 8, :], xq_sb[0:8, :])
                xqd_sb = xpool.tile([DOF, SCS], bf16, tag="xqd")
                nc.gpsimd.dma_start(xqd_sb[:, :], xqd[:, x0:x0 + SCS])

                out_sb = opool.tile([28, SCS // 4], f32, tag="out")

                for g in range(NT // 4):
                    e4_ps = pm.tile([128, TS], f32, tag="e4")
                    qg4_ps = pm.tile([128, TS], f32, tag="qg4")
                    for jj in range(4):
                        toff = (g * 4 + jj) * TS
                        tsl = slice(toff, toff + TS)
                        # --- L1: 4 row-band-packed matmuls, K=8, M=128 ---
                        h_ps = ph.tile([128, 2 * TS], f32, tag="h")
                        h2_ps = ph.tile([128, 2 * TS], f32, tag="h2")
                        for c in range(4):
                            dst = h_ps if c < 2 else h2_ps
                            nc.tensor.matmul(
                                dst[:, ts(c % 2, TS)],
                                w1_sb[32 * c:32 * c + 8, ts(c, 128)],
                                xq_sb[32 * c:32 * c + 8, tsl],
                                start=True, stop=True,
                                tile_position=(32 * c, 0),
                            )
                        # --- relu + cast to bf16: split ACT / DVE ---
                        h_sb = wpool.tile([128, 4 * TS], bf16, tag="hsb")
                        nc.scalar.activation(h_sb[:, 0:2 * TS], h_ps[:, :], AF.Relu)
                        nc.vector.tensor_scalar_max(
                            h_sb[:, 2 * TS:4 * TS], h2_ps[:, :], 0.0)
                        # --- L2: 4 accumulating matmuls, col-packed at 32*jj ---
                        for c in range(4):
                            nc.tensor.matmul(
                                e4_ps[32 * jj:32 * jj + 32, :],
                                w2_sb[:, ts(c, 32)],
                                h_sb[:, ts(c, TS)],
                                start=(c == 0), stop=(c == 3),
                                tile_position=(0, 32 * jj),
                            )
                        # --- qDD gather: K=7, M=32, col-packed ---
                        nc.tensor.matmul(
                            qg4_ps[32 * jj:32 * jj + 32, :],
                            mqg_sb[:, :], xqd_sb[:, tsl],
                            start=True, stop=True,
                            tile_position=(0, 32 * jj),
                        )
                    # --- structure stage on the packed 4-tile group ---
                    e4_sb = wpool.tile([128, TS], bf16, tag="esb")
                    nc.scalar.activation(e4_sb[:, :], e4_ps[:, :], AF.Identity,
                                         bias=b2_sb[:, :])
                    w4_sb = wpool.tile([128, TS], bf16, tag="wsb")
                    nc.vector.tensor_mul(w4_sb[:, :], e4_sb[:, :], qg4_ps[:, :])
                    vg4_ps = pm.tile([128, TS], f32, tag="vgtau")
                    nc.tensor.matmul(vg4_ps[:, :], mvg_sb[:, :], w4_sb[:, :],
                                     start=True, stop=True)
                    u4_sb = wpool.tile([128, TS], bf16, tag="usb")
                    nc.vector.tensor_mul(u4_sb[:, :], vg4_ps[:, :], e4_sb[:, :])
                    tau4_ps = pm.tile([128, TS], f32, tag="vgtau")
                    nc.tensor.matmul(tau4_ps[0:28, :], mtau_sb[:, :], u4_sb[:, :],
                                     start=True, stop=True)
                    nc.scalar.activation(out_sb[:, ts(g, TS)], tau4_ps[0:28, :],
                                         AF.Copy)
                # output super-chunk
                nc.sync.dma_start(outT[:, sc * (SCS // 4):(sc + 1) * (SCS // 4)],
                                  out_sb[:, :])
    nc.compile()
    return nc


def kernel(x, W1d, b1d, W2d, b2d, W1o, b1o, W2o, b2o):
    from concourse.bass_utils import run_bass_kernel_spmd

    key = "nc"
    if key not in _cache:
        _cache[key] = _build_nc()
    nc = _cache[key]

    wts = _build_weights(W1d, b1d, W2d, b2d, W1o, b1o, W2o, b2o)
    x = np.asarray(x, np.float32)
    in_maps = []
    for c in range(NCORES):
        sl = x[c * BS:(c + 1) * BS]
        xqc = np.empty((8, BS), np.float32)
        xqc[0:DOF] = sl[:, 0:DOF].T
        xqc[DOF] = 1.0
        in_maps.append(dict(
            xq=xqc,
            xqd=np.ascontiguousarray(sl[:, 2 * DOF:3 * DOF].T),
            **wts,
        ))
    res = run_bass_kernel_spmd(nc, in_maps, core_ids=list(range(NCORES)))
    _cache["last_results"] = res
    out = np.empty((B, DOF), np.float32)
    for c in range(NCORES):
        o = np.asarray(res.results[c]["outT"], np.float32)   # [28, BS//4]
        # row 7j+i, col G*512+s  ->  sample (4G+j)*512+s, feature i
        out[c * BS:(c + 1) * BS] = (o.reshape(4, DOF, BS // 2048, TS)
                                    .transpose(2, 0, 3, 1).reshape(BS, DOF))
    return out

